# revision 1
# baseline (speedup 1.0000x reference)
"""Trainium2 Bass kernel for SAGAN-style spatial self-attention.

Reference computation (per batch b):
    xf = x[b].reshape(C, N)                    # C=256, N=64*64=4096
    f  = w1 @ xf                               # [32, N]   (query^T)
    g  = w2 @ xf                               # [32, N]   (key)
    V  = (w3 @ xf)^T                           # [N, C]    (value)
    S  = f^T @ g                               # [N, N]
    O  = softmax(S, axis=-1) @ V               # [N, C]
    out[b] = O^T.reshape(C, H, W) + x[b]

Sharding: 8 cores = 4 batches x 2 query-halves; each core computes attention
for 2048 query positions against all 4096 keys. No cross-core communication.

Per-core algorithm:
  - key order is freely permuted (softmax sums over all m): the host packs
    this core's query chunks first so the f-projection starts early.
  - f is computed REPLICATED on all four 32-partition strips (col-tiled
    projection); g with m-sub-tile t on strip t. S^T rounds then run four
    K=32 matmuls concurrently via tile_position=(32t, 0)  (~3x vs serial).
  - S^T PSUM tiles hold 1536 elems/partition -> one ACTIVATE(Exp) each
    (amortizes the ~293ns ACT ramp); ACT writes Pt bf16 directly. A dummy
    fp32-PSUM exp during warmup hoists the 2.7us table load. (NB: a dummy
    with bf16-SBUF input leaves ACT ~20% slower for the whole run!)
  - V [4096, 257] bf16 with a ones column -> PV emits the softmax
    denominator free; O accumulated over 32 m-tiles in PSUM.
  - posts: r = 1/den; out = O*r + x (residual) in fp16.

Scheduling notes (hard-won):
  - PE clock (HAM) drops to 4/8 after ~1us idle and needs ~3.4us gapless
    streaming to recover: warmup starts immediately and the projection loop
    is ordered g -> V -> S^T round so ACT-paced stalls can't starve it.
  - ACT(exp, ~67us busy) is the serial wall: rounds are front-loaded by
    ~one chunk (global round queue) so ACT never waits at pass boundaries.
  - DMA: per-key-chunk transfers spread over the three DMA-capable rings
    (sync/scalar/gpsimd; one ring sustains only ~100GB/s) in need-order --
    SDMA round-robins rings at packet granularity, so coarse DMAs all
    finish late together. Each DMA pays a ~3us completion-semaphore tax.
    xqt is gated behind the projections to keep head bandwidth for xkv.
"""

import sys

sys.path.insert(0, "/opt/trn_rl_repo")

from contextlib import ExitStack

import numpy as np

import concourse.bass as bass
import concourse.tile as tile
from concourse import bacc, mybir
from concourse.bass import ts, ds
from concourse.bass_utils import run_bass_kernel_spmd

F32 = mybir.dt.float32
F16 = mybir.dt.float16
BF16 = mybir.dt.bfloat16

B, C, H, W = 4, 256, 64, 64
N = H * W          # 4096 keys per batch
NQ = N // 2        # 2048 queries per core
CK = 32            # query/key head dim
MT = N // 128      # 32 m-tiles
NKC = N // 512      # 8 key chunks
EXP = mybir.ActivationFunctionType.Exp

# query chunks: (column start, width)
CHUNKS = [(0, 512), (512, 512), (1024, 512), (1536, 512)]
NCH = len(CHUNKS)

# S^T PSUM tiles hold 1536 elems/partition: 3 m-slots at W=512, 6 at W=256.
# round -> tiles whose slots are complete after it (per width).
ACT_AFTER_ROUND = {
    512: {0: [0], 1: [1], 2: [2, 3], 3: [4], 4: [5], 5: [6, 7], 6: [8],
          7: [9, 10]},
    256: {1: [0], 2: [1], 4: [2], 5: [3], 7: [4, 5]},
}
SLOTS_PER_TILE = {512: 3, 256: 6}


def build_nc():
    nc = bacc.Bacc("TRN2", target_bir_lowering=False, debug=False, num_devices=8)
    xkv_d = nc.dram_tensor("xkv", [128, NKC, 2, 512], F16, kind="ExternalInput")
    xqt_d = nc.dram_tensor("xqt", [128, 16, 256], F16, kind="ExternalInput")
    wz_d = nc.dram_tensor("wz", [128, 640], F16, kind="ExternalInput")
    out_d = nc.dram_tensor("out", [128, 16, 256], F16, kind="ExternalOutput")

    with tile.TileContext(nc) as tc, ExitStack() as ctx:
        _body(ctx, tc, xkv_d.ap(), xqt_d.ap(), wz_d.ap(), out_d.ap())
    nc.compile()
    return nc


def _body(ctx, tc, xkv_d, xqt_d, wz_d, out_d):
    nc = tc.nc
    singles = ctx.enter_context(tc.tile_pool(name="singles", bufs=1))

    xkv = singles.tile([128, NKC, 2, 512], F16, tag="xkv", name="xkv")
    xqt = singles.tile([128, 16, 256], F16, tag="xqt", name="xqt")
    wz = singles.tile([128, 640], F16, tag="wz", name="wz")
    f4 = singles.tile([128, NQ], F16, tag="f4", name="f4")
    g4 = singles.tile([128, NKC, 128], F16, tag="g4", name="g4")
    V = singles.tile([128, MT, 260], BF16, tag="V", name="V")
    warm = singles.tile([128, 512], BF16, tag="warm", name="warm")
    # all allocations stay multiples of 64B/partition: a misaligned tile
    # here shifts Pt off 64B alignment and costs ~20% on ACT and PV LDWEIGHTS
    scr = singles.tile([128, 64], BF16, tag="scr", name="scr")

    # split so the first warm LDWEIGHTS (reads cols 0:128) fires before the
    # full-tile memset completes
    nc.vector.memset(warm[:, 0:128], 0.0)
    nc.vector.memset(warm[:, 128:512], 0.0)
    nc.gpsimd.memset(V[:, :, 256:257], 1.0)

    # PSUM: S^T pool 2 x 3 banks + 1-bank pool (PV accumulators, projections,
    # warmup) 2 x 1 bank = 8 banks.
    stp = ctx.enter_context(tc.tile_pool(name="st_ps", bufs=2, space="PSUM"))
    op = ctx.enter_context(tc.tile_pool(name="o_ps", bufs=2, space="PSUM"))
    # Pt triple-buffered: chunk c-1 is read by PV while chunk c is written
    # and chunk c+1's first rounds start near the end of the pass.
    ptp = ctx.enter_context(tc.tile_pool(name="pt", bufs=3))
    stgp = ctx.enter_context(tc.tile_pool(name="stage", bufs=2))
    osbp = ctx.enter_context(tc.tile_pool(name="osb", bufs=2))
    rp = ctx.enter_context(tc.tile_pool(name="r", bufs=2))

    w1t = [wz[:, 320 * k:320 * k + 32] for k in range(2)]
    w2t = [wz[:, 320 * k + 32:320 * k + 64] for k in range(2)]
    w3t = [wz[:, 320 * k + 64:320 * k + 320] for k in range(2)]

    Pt = {}
    posts = []
    st_tiles = {}   # (chunk, q) -> psum tile

    def emit_post(item):
        ci, j, o_ps, stg = item
        start, width = CHUNKS[ci]
        nj = width // 128
        J = start // 128 + j
        r = rp.tile([128, 16], F32, tag="r", name="r")
        nc.vector.reciprocal(r[:, 0:1], o_ps[:, 256:257])
        o_sb = osbp.tile([128, 256], F16, tag="osb", name="osb")
        nc.vector.tensor_scalar_mul(o_sb[:], o_ps[:, 0:256], r[:, 0:1])
        nc.vector.tensor_add(stg[:, j, :], o_sb[:], xqt[:, J, :])
        J0 = start // 128
        if ci == NCH - 1:
            # stream the final chunk's output per j-tile so the last DMA
            # (and its ~3us completion tax) starts as early as possible
            engs = [nc.sync, nc.gpsimd, nc.scalar]
            engs[j % 3].dma_start(out_d[:, J0 + j:J0 + j + 1, :],
                                  stg[:, j:j + 1, :])
        elif j == nj - 1:
            nc.gpsimd.dma_start(out_d[:, J0:J0 + nj, :], stg[:, 0:nj, :])

    def st_round(ci, r):
        # Row-tiled round: 4 concurrent K=32 matmuls for m-tiles 4r..4r+3 of
        # query chunk ci; strip t holds g for m-sub-tile t of key chunk r.
        start, width = CHUNKS[ci]
        spt = SLOTS_PER_TILE[width]
        for t in range(4):
            s = 4 * r + t
            q, sub = divmod(s, spt)
            if sub == 0:
                st_tiles[(ci, q)] = stp.tile([128, spt, width], F32, tag="st",
                                             name="st")
            nc.tensor.matmul(st_tiles[(ci, q)][:, sub, :],
                             g4[32 * t:32 * t + 32, r, :],
                             f4[32 * t:32 * t + 32, ds(start, width)],
                             start=True, stop=True, tile_position=(32 * t, 0),
                             skip_group_check=True)

    def emit_acts(ci, r):
        width = CHUNKS[ci][1]
        spt = SLOTS_PER_TILE[width]
        for q in ACT_AFTER_ROUND[width].get(r, []):
            nslot = min(spt, MT - spt * q)
            tl = st_tiles.pop((ci, q))
            nc.scalar.activation(Pt[ci][:, spt * q:spt * q + nslot, :],
                                 tl[:, 0:nslot, :], EXP)

    def alloc_pt(ci):
        Pt[ci] = ptp.tile([128, MT, CHUNKS[ci][1]], BF16, tag="pt", name="pt")

    def fproj(qc):
        fp = op.tile([128, 512], F32, tag="o", name="fp")
        for k in range(2):
            for j in range(4):
                nc.tensor.matmul(fp[32 * j:32 * j + 32, :], w1t[k],
                                 xkv[:, qc, k, :], start=(k == 0),
                                 stop=(k == 1), tile_position=(0, 32 * j),
                                 skip_group_check=True)
        nc.vector.tensor_copy(f4[:, ts(qc, 512)], fp[:])

    # ---- input DMAs first: per key chunk, three rings, need-order (a single
    # ring sustains only ~100GB/s; HBM allows ~358) ----
    nc.gpsimd.dma_start(wz[:], wz_d[:, :])
    # chunks 0 and 1 gate the first projections: split their k-halves across
    # both HW rings so they complete first (in need-order)
    for i in range(2):
        nc.sync.dma_start(xkv[:, i, 0, :], xkv_d[:, i, 0, :])
        nc.scalar.dma_start(xkv[:, i, 1, :], xkv_d[:, i, 1, :])
    for i, eng in [(2, nc.sync), (3, nc.gpsimd), (4, nc.scalar), (5, nc.sync),
                   (6, nc.gpsimd), (7, nc.scalar)]:
        eng.dma_start(xkv[:, i, :, :], xkv_d[:, i, :, :])

    # HAM warmup: keep the PE streaming while the first input DMAs land so
    # the clock gate opens before the projection phase.
    wps = [op.tile([128, 512], F32, tag="o", name="wps") for _ in range(2)]
    for i in range(16):
        nc.tensor.matmul(wps[i % 2][:], warm[:, 0:128], warm[:],
                         start=True, stop=True)

    # Dummy exp (same fp32-PSUM -> bf16-SBUF shape as the real calls) to
    # hoist the ~2.7us ACT table load into the warmup window.
    nc.scalar.activation(scr[:, 0:1], wps[0][:, 0:1], EXP)

    # ---- projections; chunk-0 S^T rounds lag g by one key chunk and sit
    # after the dense V work so ACT-paced stalls never break the PE stream ----
    alloc_pt(0)
    alloc_pt(1)
    fproj(0)
    fproj(1)
    for i in range(NKC):
        gp = op.tile([128, 128], F32, tag="o", name="gp")
        for k in range(2):
            for t in range(4):
                nc.tensor.matmul(gp[32 * t:32 * t + 32, :], w2t[k],
                                 xkv[:, i, k, ts(t, 128)], start=(k == 0),
                                 stop=(k == 1), tile_position=(0, 32 * t),
                                 skip_group_check=True)
        nc.vector.tensor_copy(g4[:, i, :], gp[:])
        for p in range(2):
            vp = op.tile([128, 2, 256], F32, tag="o", name="vp")
            for u in range(2):
                for k in range(2):
                    nc.tensor.matmul(vp[:, u, :], xkv[:, i, k, ts(2 * p + u, 128)],
                                     w3t[k], start=(k == 0), stop=(k == 1))
            nc.vector.tensor_copy(V[:, 4 * i + 2 * p:4 * i + 2 * p + 2, 0:256],
                                  vp[:])
        if i == 2:
            fproj(2)
            fproj(3)
        if i >= 1:
            st_round(0, i - 1)
            emit_acts(0, i - 1)
    st_round(0, NKC - 1)
    emit_acts(0, NKC - 1)
    # first rounds of chunk 1 keep ACT busy across the seam
    for r in range(4):
        st_round(1, r)
        emit_acts(1, r)

    # xqt (residual) is first needed by the posts of pass 1 (~45us); gate its
    # DMA on a mid-projection tile so it doesn't steal head bandwidth from xkv
    nc.gpsimd.tensor_copy(scr[:, 32:48], g4[:, 3, 0:16])
    for jh in range(2):
        nc.gpsimd.dma_start(xqt[:, 8 * jh:8 * jh + 8, :],
                            xqt_d[:, 8 * jh:8 * jh + 8, :])

    # global round queue: remaining rounds in order, popped at pass slots
    RQ = [(1, r) for r in range(4, 8)]
    for ci in range(2, NCH):
        RQ += [(ci, r) for r in range(8)]

    # ---- attention passes: pass p does PV for chunk p-1 and pops S^T
    # rounds (front-loaded by ~one chunk) at every other PV segment ----
    for p in range(1, NCH + 1):
        ci = p - 1
        start, width = CHUNKS[ci]
        nj = width // 128
        stg = stgp.tile([128, nj, 256], F16, tag="stage", name="stage")
        for j in range(nj):
            o_cur = op.tile([128, 257], F32, tag="o", name="o")
            for seg in range(4):
                for mm in range(4):
                    mt = seg * 8 + mm
                    nc.tensor.matmul(o_cur[:], Pt[ci][:, mt, ts(j, 128)],
                                     V[:, mt, 0:257],
                                     start=(mt == 0), stop=(mt == MT - 1),
                                     skip_group_check=True)
                # front-load all rounds into j=0,1: ACT builds a deep queue
                # early in the pass and never starves at pass boundaries
                if j < 2 and RQ:
                    rc, rr = RQ.pop(0)
                    if rr == 0:
                        alloc_pt(rc)
                    st_round(rc, rr)
                    emit_acts(rc, rr)
                for mm in range(4, 8):
                    mt = seg * 8 + mm
                    nc.tensor.matmul(o_cur[:], Pt[ci][:, mt, ts(j, 128)],
                                     V[:, mt, 0:257],
                                     start=(mt == 0), stop=(mt == MT - 1),
                                     skip_group_check=True)
            posts.append((ci, j, o_cur, stg))
            # delay each n-tile's post-processing by one PE group so the DVE
            # normalize never stalls the PE stream; the final pass has no
            # S^T stream left to protect, so flush immediately there
            while len(posts) > (1 if (j < nj - 1 and p < NCH) else 0):
                emit_post(posts.pop(0))
    while posts:
        emit_post(posts.pop(0))


_NC_CACHE = None


def _get_nc():
    global _NC_CACHE
    if _NC_CACHE is None:
        _NC_CACHE = build_nc()
    return _NC_CACHE


def make_in_maps(x, w1, w2, w3):
    x = np.ascontiguousarray(x, dtype=np.float32).reshape(B, C, N)
    xh = x.astype(np.float16)
    # weights packed [128, 640]: per k-half, cols 0:32 w1^T, 32:64 w2^T,
    # 64:320 w3^T (k=0 at 0:320, k=1 at 320:640) -> one contiguous DMA run
    wz = np.empty((128, 640), dtype=np.float16)
    for k in range(2):
        wz[:, 320 * k:320 * k + 32] = w1.T[128 * k:128 * k + 128, :]
        wz[:, 320 * k + 32:320 * k + 64] = w2.T[128 * k:128 * k + 128, :]
        wz[:, 320 * k + 64:320 * k + 320] = w3.T[128 * k:128 * k + 128, :]
    in_maps = []
    for core in range(8):
        b, half = core // 2, core % 2
        # key-chunk permutation: this core's query chunks first
        perm = [4 * half + i for i in range(4)] + \
               [4 * (1 - half) + i for i in range(4)]
        # xkv packed [128, 8, 2, 512]: [p, ch, k, s] = xh[b][128k+p, 512*perm[ch]+s]
        xv = xh[b].reshape(2, 128, NKC, 512).transpose(1, 2, 0, 3)[:, perm]
        # residual x^T for this core's queries: [2048, 256] -> [128, 16, 256]
        xq = xh[b][:, half * NQ:(half + 1) * NQ].T
        xqt = xq.reshape(16, 128, 256).transpose(1, 0, 2)
        in_maps.append({
            "xkv": np.ascontiguousarray(xv),
            "xqt": np.ascontiguousarray(xqt),
            "wz": wz,
        })
    return in_maps


def assemble(results):
    out = np.empty((B, C, N), dtype=np.float32)
    for core in range(8):
        b, half = core // 2, core % 2
        o = results[core]["out"].astype(np.float32)      # [128, 16, 256]
        o = o.transpose(1, 0, 2).reshape(NQ, C)          # [(J p), c]
        out[b][:, half * NQ:(half + 1) * NQ] = o.T
    return out.reshape(B, C, H, W)


def kernel(x, w1, w2, w3):
    nc = _get_nc()
    res = run_bass_kernel_spmd(nc, make_in_maps(x, w1, w2, w3),
                               core_ids=list(range(8)))
    return assemble(res.results)

